# revision 1
# baseline (speedup 1.0000x reference)
"""Trainium2 Bass kernel for nn_CircularBlur: depthwise 4x4 blur with
circular padding on (4, 512, 256, 256) fp32.

Math (derived from the reference's wrap-pad + zero-pad + flipped-kernel
conv + crop; the zero padding never reaches the cropped region):

    out[n,c,y,x] = sum_{i,j} k[i,j] * in[n,c,(y+1-i)%256,(x+1-j)%256]

Strategy: pure data parallel over the 2048 (n,c) images, 256 per core.
Per image the blur is separable (k = a outer b via SVD).  The vertical
pass is a banded-circulant matmul on the tensor engine (stationary =
128x128 chunks of V^T, prescaled by the horizontal tap weights).  The
horizontal taps become shifted column windows of the moving operand;
symmetric tap pairs are pre-summed on the vector engine so each pair
costs one matmul instead of two.  Column wrap is handled with a 3-col
halo filled by on-chip copies; row wrap is baked into V.
"""

import sys

sys.path.insert(0, "/opt/trn_rl_repo")

import numpy as np

N_CORES = 8
H = W = 256
HALO = W + 3  # 2 left wrap cols + 256 + 1 right wrap col
IMG_TOTAL = 4 * 512
IMG_PER_CORE = IMG_TOTAL // N_CORES  # 256
G = 8  # images per group (2MB per DMA)
NGROUPS = IMG_PER_CORE // G
KH = KW = 4


def _decompose(k):
    """k (4,4) float64 -> list of rank-1 terms [(a[4], taps)], where
    taps = [(scale, [shifts...])]; shift s means column x+s contributes
    with weight scale (after pre-summing all shifts in the group)."""
    U, S, Vt = np.linalg.svd(k)
    terms = []
    for r in range(KH):
        if S[r] <= max(S[0] * 1e-7, 1e-30):
            continue
        a = U[:, r] * np.sqrt(S[r])
        b = Vt[r] * np.sqrt(S[r])
        # tap j has shift 1-j and weight b[j]
        tol = 1e-9 * max(1.0, np.abs(b).max())
        if abs(b[0] - b[3]) <= tol and abs(b[1] - b[2]) <= tol:
            taps = [(b[0], [1, -2]), (b[1], [0, -1])]
        else:
            taps = [(b[j], [1 - j]) for j in range(KW)]
        terms.append((a, taps))
    return terms


def _build_weights(terms):
    """Host-side stationary blocks.  Returns (W_host [128, NIDX, 128] f32,
    mov_shifts: list of shift-lists, one per moving tensor)."""
    movs = []  # (a_vec, scale, shifts)
    for a, taps in terms:
        for scale, shifts in taps:
            movs.append((a, scale, shifts))
    n_idx = len(movs) * 4
    Wh = np.zeros((128, n_idx, 128), np.float32)
    yy = np.arange(H)
    for mi, (a, scale, _shifts) in enumerate(movs):
        V = np.zeros((H, H), np.float64)
        for i in range(KH):
            V[yy, (yy + 1 - i) % H] += a[i]
        VT = (scale * V).T  # VT[v, y]
        for kc in range(2):
            for yb in range(2):
                idx = (mi * 2 + kc) * 2 + yb
                Wh[:, idx, :] = VT[kc * 128:(kc + 1) * 128,
                                   yb * 128:(yb + 1) * 128].astype(np.float32)
    return Wh, [m[2] for m in movs]


_PROGRAM_CACHE = {}


def _build_program(mov_shifts):
    """Build + compile the per-core Bass program.  mov_shifts: list of
    shift-lists (structure only; weights arrive via the `w` input)."""
    import concourse.bacc as bacc
    import concourse.mybir as mybir
    from concourse import tile

    key = tuple(tuple(s) for s in mov_shifts)
    if key in _PROGRAM_CACHE:
        return _PROGRAM_CACHE[key]

    f32 = mybir.dt.float32
    n_movs = len(mov_shifts)
    n_idx = n_movs * 4

    nc = bacc.Bacc("TRN2", target_bir_lowering=False, debug=False,
                   num_devices=N_CORES)
    x_in = nc.declare_dram_parameter("x", [IMG_PER_CORE, H, W], f32,
                                     isOutput=False)
    w_in = nc.declare_dram_parameter("w", [128, n_idx, 128], f32,
                                     isOutput=False)
    y_out = nc.declare_dram_parameter("y", [IMG_PER_CORE, H, W], f32,
                                      isOutput=True)

    with tile.TileContext(nc) as tc:
        with (
            tc.tile_pool(name="const", bufs=1) as cpool,
            tc.tile_pool(name="xin", bufs=2) as xpool,
            tc.tile_pool(name="mov", bufs=2) as mpool,
            tc.tile_pool(name="outp", bufs=2) as opool,
            tc.tile_pool(name="psum", bufs=4, space="PSUM") as pspool,
        ):
            wt = cpool.tile([128, n_idx, 128], f32)
            nc.sync.dma_start(wt[:], w_in[:])

            for g in range(NGROUPS):
                gs = slice(g * G, (g + 1) * G)
                x2 = xpool.tile([128, G, 2, HALO], f32, tag="x2")
                nc.sync.dma_start(
                    x2[:, :, :, 2:2 + W],
                    x_in[gs].rearrange("m (r p) w -> p m r w", p=128),
                )
                # column wrap halo: cols [0,2) <- img cols 254,255 ; col 258 <- img col 0
                nc.vector.tensor_copy(x2[:, :, :, 0:2], x2[:, :, :, W:W + 2])
                nc.vector.tensor_copy(x2[:, :, :, W + 2:W + 3], x2[:, :, :, 2:3])

                movs = []
                for ti, shifts in enumerate(mov_shifts):
                    if len(shifts) == 1:
                        movs.append((x2, shifts[0] + 2))
                    else:
                        pt = mpool.tile([128, G, 2, W], f32, tag=f"p{ti}")
                        s0, s1 = shifts[0], shifts[1]
                        nc.vector.tensor_add(
                            pt[:],
                            x2[:, :, :, s0 + 2:s0 + 2 + W],
                            x2[:, :, :, s1 + 2:s1 + 2 + W],
                        )
                        for s_extra in shifts[2:]:
                            nc.vector.tensor_add(
                                pt[:], pt[:],
                                x2[:, :, :, s_extra + 2:s_extra + 2 + W],
                            )
                        movs.append((pt, 0))

                yt = opool.tile([128, G, 2, W], f32, tag="yt")
                for pr in range(G // 2):
                    for yb in range(2):
                        ps = pspool.tile([128, 2, W], f32, tag="ps")
                        mms = [(mi, kc) for mi in range(n_movs)
                               for kc in range(2)]
                        for q, (mi, kc) in enumerate(mms):
                            idx = (mi * 2 + kc) * 2 + yb
                            src, c0 = movs[mi]
                            rhs = src[:, 2 * pr:2 * pr + 2, kc, c0:c0 + W]
                            nc.tensor.matmul(
                                ps[:], wt[:, idx, :], rhs,
                                start=(q == 0), stop=(q == len(mms) - 1),
                            )
                        nc.scalar.copy(yt[:, 2 * pr:2 * pr + 2, yb, :], ps[:])

                nc.sync.dma_start(
                    y_out[gs].rearrange("m (r p) w -> p m r w", p=128),
                    yt[:],
                )

    nc.compile()
    _PROGRAM_CACHE[key] = nc
    return nc


def kernel(input, kernel):
    input = np.ascontiguousarray(np.asarray(input, dtype=np.float32))
    k = np.asarray(kernel, dtype=np.float64)
    assert input.shape == (4, 512, H, W) and k.shape == (KH, KW)

    terms = _decompose(k)
    if not terms:
        return np.zeros_like(input)

    Wh, mov_shifts = _build_weights(terms)
    nc = _build_program(mov_shifts)

    from concourse.bass_utils import run_bass_kernel_spmd

    x_flat = input.reshape(IMG_TOTAL, H, W)
    in_maps = [
        {"x": x_flat[c * IMG_PER_CORE:(c + 1) * IMG_PER_CORE], "w": Wh}
        for c in range(N_CORES)
    ]
    res = run_bass_kernel_spmd(nc, in_maps, list(range(N_CORES)))
    out = np.concatenate([res.results[c]["y"] for c in range(N_CORES)], axis=0)
    return out.reshape(4, 512, H, W).astype(np.float32, copy=False)


# revision 4
# speedup vs baseline: 1.9896x; 1.9896x over previous
"""Trainium2 Bass kernel for nn_CircularBlur: depthwise 4x4 blur with
circular padding on (4, 512, 256, 256) fp32.

Math (derived from the reference's wrap-pad + zero-pad + flipped-kernel
conv + crop; the zero padding never reaches the cropped region):

    out[n,c,y,x] = sum_{i,j} k[i,j] * in[n,c,(y+1-i)%256,(x+1-j)%256]

Strategy: pure data parallel over the 2048 (n,c) images, 256 per core.
Per image the blur is separable (k = a outer b via SVD).  The vertical
pass is a banded-circulant matmul on the tensor engine (stationary =
128x128 chunks of V^T, prescaled by the horizontal tap weights).  The
horizontal taps become shifted column windows of the moving operand;
symmetric tap pairs are pre-summed on the vector engine so each pair
costs one matmul instead of two.  Column wrap is handled with a 3-col
halo filled by on-chip copies; row wrap is baked into V.
"""

import sys

sys.path.insert(0, "/opt/trn_rl_repo")

import numpy as np

N_CORES = 8
H = W = 256
HALO = W + 3  # 2 left wrap cols + 256 + 1 right wrap col
IMG_TOTAL = 4 * 512
IMG_PER_CORE = IMG_TOTAL // N_CORES  # 256
G = 8  # images per group (2MB per DMA)
NGROUPS = IMG_PER_CORE // G
KH = KW = 4


def _decompose(k):
    """k (4,4) float64 -> list of rank-1 terms [(a[4], taps)], where
    taps = [(scale, [shifts...])]; shift s means column x+s contributes
    with weight scale (after pre-summing all shifts in the group)."""
    U, S, Vt = np.linalg.svd(k)
    terms = []
    for r in range(KH):
        if S[r] <= max(S[0] * 1e-7, 1e-30):
            continue
        a = U[:, r] * np.sqrt(S[r])
        b = Vt[r] * np.sqrt(S[r])
        # tap j has shift 1-j and weight b[j]
        tol = 1e-9 * max(1.0, np.abs(b).max())
        if abs(b[0] - b[3]) <= tol and abs(b[1] - b[2]) <= tol:
            taps = [(b[0], [1, -2]), (b[1], [0, -1])]
        else:
            taps = [(b[j], [1 - j]) for j in range(KW)]
        terms.append((a, taps))
    return terms


def _build_weights(terms):
    """Host-side stationary blocks.  Returns (W_host [128, NIDX, 128] f32,
    mov_shifts: list of shift-lists, one per moving tensor)."""
    movs = []  # (a_vec, scale, shifts)
    for a, taps in terms:
        for scale, shifts in taps:
            movs.append((a, scale, shifts))
    n_idx = len(movs) * 4
    Wh = np.zeros((128, n_idx, 128), np.float32)
    yy = np.arange(H)
    for mi, (a, scale, _shifts) in enumerate(movs):
        V = np.zeros((H, H), np.float64)
        for i in range(KH):
            V[yy, (yy + 1 - i) % H] += a[i]
        VT = (scale * V).T  # VT[v, y]
        for kc in range(2):
            for yb in range(2):
                idx = (mi * 2 + kc) * 2 + yb
                Wh[:, idx, :] = VT[kc * 128:(kc + 1) * 128,
                                   yb * 128:(yb + 1) * 128].astype(np.float32)
    return Wh, [m[2] for m in movs]


_PROGRAM_CACHE = {}


def _build_program(mov_shifts):
    """Build + compile the per-core Bass program.  mov_shifts: list of
    shift-lists (structure only; weights arrive via the `w` input)."""
    import concourse.bacc as bacc
    import concourse.mybir as mybir
    from concourse import tile

    key = tuple(tuple(s) for s in mov_shifts)
    if key in _PROGRAM_CACHE:
        return _PROGRAM_CACHE[key]

    f32 = mybir.dt.float32
    f32r = mybir.dt.float32r
    n_movs = len(mov_shifts)
    n_idx = n_movs * 4

    nc = bacc.Bacc("TRN2", target_bir_lowering=False, debug=False,
                   num_devices=N_CORES)
    x_in = nc.declare_dram_parameter("x", [IMG_PER_CORE, H, W], f32r,
                                     isOutput=False)
    w_in = nc.declare_dram_parameter("w", [128, n_idx, 128], f32r,
                                     isOutput=False)
    y_out = nc.declare_dram_parameter("y", [IMG_PER_CORE, H, W], f32,
                                      isOutput=True)

    with tile.TileContext(nc) as tc:
        with (
            tc.tile_pool(name="const", bufs=1) as cpool,
            tc.tile_pool(name="xin", bufs=2) as xpool,
            tc.tile_pool(name="mov", bufs=2) as mpool,
            tc.tile_pool(name="outp", bufs=2) as opool,
            tc.tile_pool(name="psum", bufs=4, space="PSUM") as pspool,
        ):
            wt = cpool.tile([128, n_idx, 128], f32r)
            nc.sync.dma_start(wt[:], w_in[:])

            for g in range(NGROUPS):
                gs = slice(g * G, (g + 1) * G)
                x2 = xpool.tile([128, G, 2, HALO], f32r, tag="x2")
                nc.sync.dma_start(
                    x2[:, :, :, 2:2 + W],
                    x_in[gs].rearrange("m (r p) w -> p m r w", p=128),
                )
                # column wrap halo: cols [0,2) <- img cols 254,255 ; col 258 <- img col 0
                nc.vector.tensor_copy(x2[:, :, :, 0:2], x2[:, :, :, W:W + 2])
                nc.vector.tensor_copy(x2[:, :, :, W + 2:W + 3], x2[:, :, :, 2:3])

                movs = []
                for ti, shifts in enumerate(mov_shifts):
                    if len(shifts) == 1:
                        movs.append((x2, shifts[0] + 2))
                    else:
                        pt = mpool.tile([128, G, 2, W], f32r, tag=f"p{ti}")
                        s0, s1 = shifts[0], shifts[1]
                        nc.vector.tensor_add(
                            pt[:],
                            x2[:, :, :, s0 + 2:s0 + 2 + W],
                            x2[:, :, :, s1 + 2:s1 + 2 + W],
                        )
                        for s_extra in shifts[2:]:
                            nc.vector.tensor_add(
                                pt[:], pt[:],
                                x2[:, :, :, s_extra + 2:s_extra + 2 + W],
                            )
                        movs.append((pt, 0))

                yt = opool.tile([128, G, 2, W], f32, tag="yt")
                for pr in range(G // 2):
                    for yb in range(2):
                        ps = pspool.tile([128, 2, W], f32, tag="ps")
                        mms = [(mi, kc) for mi in range(n_movs)
                               for kc in range(2)]
                        for q, (mi, kc) in enumerate(mms):
                            idx = (mi * 2 + kc) * 2 + yb
                            src, c0 = movs[mi]
                            rhs = src[:, 2 * pr:2 * pr + 2, kc, c0:c0 + W]
                            # float32r streams 1 col/cycle (vs 4 for plain
                            # fp32) at matmul free dim >= 256
                            nc.tensor.matmul(
                                ps[:], wt[:, idx, :], rhs,
                                start=(q == 0), stop=(q == len(mms) - 1),
                            )
                        nc.scalar.copy(yt[:, 2 * pr:2 * pr + 2, yb, :], ps[:])

                nc.sync.dma_start(
                    y_out[gs].rearrange("m (r p) w -> p m r w", p=128),
                    yt[:],
                )

    nc.compile()
    _PROGRAM_CACHE[key] = nc
    return nc


def kernel(input, kernel):
    input = np.ascontiguousarray(np.asarray(input, dtype=np.float32))
    k = np.asarray(kernel, dtype=np.float64)
    assert input.shape == (4, 512, H, W) and k.shape == (KH, KW)

    terms = _decompose(k)
    if not terms:
        return np.zeros_like(input)

    Wh, mov_shifts = _build_weights(terms)
    nc = _build_program(mov_shifts)

    from concourse.bass_utils import run_bass_kernel_spmd

    x_flat = input.reshape(IMG_TOTAL, H, W)
    in_maps = [
        {"x": x_flat[c * IMG_PER_CORE:(c + 1) * IMG_PER_CORE], "w": Wh}
        for c in range(N_CORES)
    ]
    res = run_bass_kernel_spmd(nc, in_maps, list(range(N_CORES)))
    out = np.concatenate([res.results[c]["y"] for c in range(N_CORES)], axis=0)
    return out.reshape(4, 512, H, W).astype(np.float32, copy=False)


# revision 7
# speedup vs baseline: 2.2294x; 1.1205x over previous
"""Trainium2 Bass kernel for nn_CircularBlur: depthwise 4x4 blur with
circular padding on (4, 512, 256, 256) fp32.

Math (derived from the reference's wrap-pad + zero-pad + flipped-kernel
conv + crop; the zero padding never reaches the cropped region):

    out[n,c,y,x] = sum_{i,j} k[i,j] * in[n,c,(y+1-i)%256,(x+1-j)%256]

Strategy: pure data parallel over the 2048 (n,c) images, 256 per core.
Per image the blur is separable (k = a outer b via SVD).  The vertical
pass is a banded-circulant matmul on the tensor engine (stationary =
128x128 chunks of V^T, prescaled by the horizontal tap weights).  The
horizontal taps become shifted column windows of the moving operand;
symmetric tap pairs are pre-summed on the vector engine so each pair
costs one matmul instead of two.  Column wrap is handled with a 3-col
halo filled by on-chip copies; row wrap is baked into V.
"""

import sys

sys.path.insert(0, "/opt/trn_rl_repo")

import numpy as np

N_CORES = 8
H = W = 256
HALO = W + 3  # 2 left wrap cols + 256 + 1 right wrap col
IMG_TOTAL = 4 * 512
IMG_PER_CORE = IMG_TOTAL // N_CORES  # 256
G = 8  # images per group (2MB per DMA)
NGROUPS = IMG_PER_CORE // G
KH = KW = 4


def _decompose(k):
    """k (4,4) float64 -> list of rank-1 terms [(a[4], taps)], where
    taps = [(scale, [shifts...])]; shift s means column x+s contributes
    with weight scale (after pre-summing all shifts in the group)."""
    U, S, Vt = np.linalg.svd(k)
    terms = []
    for r in range(KH):
        if S[r] <= max(S[0] * 1e-7, 1e-30):
            continue
        a = U[:, r] * np.sqrt(S[r])
        b = Vt[r] * np.sqrt(S[r])
        # tap j has shift 1-j and weight b[j]
        tol = 1e-9 * max(1.0, np.abs(b).max())
        if abs(b[0] - b[3]) <= tol and abs(b[1] - b[2]) <= tol:
            taps = [(b[0], [1, -2]), (b[1], [0, -1])]
        else:
            taps = [(b[j], [1 - j]) for j in range(KW)]
        terms.append((a, taps))
    return terms


def _build_weights(terms):
    """Host-side stationary blocks.  Returns (W_host [128, NIDX, 128] f32,
    mov_shifts: list of shift-lists, one per moving tensor)."""
    movs = []  # (a_vec, scale, shifts)
    for a, taps in terms:
        for scale, shifts in taps:
            movs.append((a, scale, shifts))
    n_idx = len(movs) * 4
    Wh = np.zeros((128, n_idx, 128), np.float32)
    yy = np.arange(H)
    for mi, (a, scale, _shifts) in enumerate(movs):
        V = np.zeros((H, H), np.float64)
        for i in range(KH):
            V[yy, (yy + 1 - i) % H] += a[i]
        VT = (scale * V).T  # VT[v, y]
        for kc in range(2):
            for yb in range(2):
                idx = (mi * 2 + kc) * 2 + yb
                # row v=2*vp+kc lives on partition vp; out row y=2*m+yb on
                # psum partition m (even/odd interleave -> 2KB DMA chunks)
                Wh[:, idx, :] = VT[kc::2, yb::2].astype(np.float32)
    return Wh, [m[2] for m in movs]


_PROGRAM_CACHE = {}


def _build_program(mov_shifts):
    """Build + compile the per-core Bass program.  mov_shifts: list of
    shift-lists (structure only; weights arrive via the `w` input)."""
    import concourse.bacc as bacc
    import concourse.mybir as mybir
    from concourse import tile

    key = tuple(tuple(s) for s in mov_shifts)
    if key in _PROGRAM_CACHE:
        return _PROGRAM_CACHE[key]

    f32 = mybir.dt.float32
    f32r = mybir.dt.float32r
    n_movs = len(mov_shifts)
    n_idx = n_movs * 4

    nc = bacc.Bacc("TRN2", target_bir_lowering=False, debug=False,
                   num_devices=N_CORES)
    x_in = nc.declare_dram_parameter("x", [IMG_PER_CORE, H, W], f32r,
                                     isOutput=False)
    w_in = nc.declare_dram_parameter("w", [128, n_idx, 128], f32r,
                                     isOutput=False)
    y_out = nc.declare_dram_parameter("y", [IMG_PER_CORE, H, W], f32,
                                      isOutput=True)

    with tile.TileContext(nc) as tc:
        with (
            tc.tile_pool(name="const", bufs=1) as cpool,
            tc.tile_pool(name="xin", bufs=3) as xpool,
            tc.tile_pool(name="mov", bufs=2) as mpool,
            tc.tile_pool(name="outp", bufs=3) as opool,
            tc.tile_pool(name="psum", bufs=4, space="PSUM") as pspool,
        ):
            wt = cpool.tile([128, n_idx, 128], f32r)
            nc.sync.dma_start(wt[:], w_in[:])

            for g in range(NGROUPS):
                gs = slice(g * G, (g + 1) * G)
                # compact (halo-free) layout: per partition the G*2*W floats
                # are contiguous, so the load DMA merges to 2KB descriptors
                xc = xpool.tile([128, G, 2, W], f32r, tag="xc")
                nc.sync.dma_start(
                    xc[:], x_in[gs].rearrange("m (p r) w -> p m r w", r=2)
                )

                movs = []
                for ti, shifts in enumerate(mov_shifts):
                    # moving tile P[x] = sum_s xc[(x+s) % W]; wrap-free main
                    # range in one op, wrapped boundary columns one op each
                    pt = mpool.tile([128, G, 2, W], f32r, tag=f"p{ti}")
                    lo = max(0, *(-s for s in shifts))
                    hi = min(W, *(W - s for s in shifts))
                    if len(shifts) == 1:
                        s = shifts[0]
                        nc.vector.tensor_copy(
                            pt[:, :, :, lo:hi], xc[:, :, :, lo + s:hi + s]
                        )
                        for x in list(range(lo)) + list(range(hi, W)):
                            c = (x + s) % W
                            nc.vector.tensor_copy(
                                pt[:, :, :, x:x + 1], xc[:, :, :, c:c + 1]
                            )
                    else:
                        assert len(shifts) == 2
                        s0, s1 = shifts[0], shifts[1]
                        nc.vector.tensor_add(
                            pt[:, :, :, lo:hi],
                            xc[:, :, :, lo + s0:hi + s0],
                            xc[:, :, :, lo + s1:hi + s1],
                        )
                        for x in list(range(lo)) + list(range(hi, W)):
                            c0 = (x + s0) % W
                            c1 = (x + s1) % W
                            nc.vector.tensor_add(
                                pt[:, :, :, x:x + 1],
                                xc[:, :, :, c0:c0 + 1],
                                xc[:, :, :, c1:c1 + 1],
                            )
                    movs.append((pt, 0))

                yt = opool.tile([128, G, 2, W], f32, tag="yt")
                for pr in range(G // 2):
                    for yb in range(2):
                        ps = pspool.tile([128, 2, W], f32, tag="ps")
                        mms = [(mi, kc) for mi in range(n_movs)
                               for kc in range(2)]
                        for q, (mi, kc) in enumerate(mms):
                            idx = (mi * 2 + kc) * 2 + yb
                            src, c0 = movs[mi]
                            rhs = src[:, 2 * pr:2 * pr + 2, kc, c0:c0 + W]
                            # float32r streams 1 col/cycle (vs 4 for plain
                            # fp32) at matmul free dim >= 256
                            nc.tensor.matmul(
                                ps[:], wt[:, idx, :], rhs,
                                start=(q == 0), stop=(q == len(mms) - 1),
                            )
                        nc.scalar.copy(yt[:, 2 * pr:2 * pr + 2, yb, :], ps[:])

                nc.sync.dma_start(
                    y_out[gs].rearrange("m (p r) w -> p m r w", r=2),
                    yt[:],
                )

    nc.compile()
    _PROGRAM_CACHE[key] = nc
    return nc


def kernel(input, kernel):
    input = np.ascontiguousarray(np.asarray(input, dtype=np.float32))
    k = np.asarray(kernel, dtype=np.float64)
    assert input.shape == (4, 512, H, W) and k.shape == (KH, KW)

    terms = _decompose(k)
    if not terms:
        return np.zeros_like(input)

    Wh, mov_shifts = _build_weights(terms)
    nc = _build_program(mov_shifts)

    from concourse.bass_utils import run_bass_kernel_spmd

    x_flat = input.reshape(IMG_TOTAL, H, W)
    in_maps = [
        {"x": x_flat[c * IMG_PER_CORE:(c + 1) * IMG_PER_CORE], "w": Wh}
        for c in range(N_CORES)
    ]
    res = run_bass_kernel_spmd(nc, in_maps, list(range(N_CORES)))
    out = np.concatenate([res.results[c]["y"] for c in range(N_CORES)], axis=0)
    return out.reshape(4, 512, H, W).astype(np.float32, copy=False)
